# revision 14
# baseline (speedup 1.0000x reference)
"""Trainium2 Bass kernel: RoPE causal attention (B=1,S=2048,D=4096,H=32).

Tensor-parallel over heads on 8 NeuronCores: core c owns heads [4c,4c+4).
Fully fused single-pass kernel, no DRAM round trips between stages:

- Host passes x pre-transposed (xT [D,S]) and weights pre-transposed in
  bf16, with wq/wk rows pair-permuted per head (evens then odds) so RoPE
  pairs land in contiguous partition halves. cos/sin are precomputed on
  host, transposed to [HD/2, S].
- Projections compute qT/kT in [hd, s] layout directly (lhsT = w tiles,
  rhs = xT tiles) and v in [s, hd] layout; RoPE is fused into the
  PSUM->SBUF evacuation (DVE mults + GpSimd add/sub).
- Attention computes scores transposed [sk, sq] so that exp(scores)
  (written by ACT straight into SBUF) is directly the lhsT of the P@V
  matmul -- no PE transposes, no probability copies. Softmax skips the
  max subtraction (|scores/sqrt(hd)| <= ~10, exp cannot overflow) and
  folds normalization in after P@V: a ones-matmul gives the row-sum
  broadcast across partitions, one reciprocal + one multiply normalize.
- wo consumes the attention output from SBUF, accumulating over the 4
  local head slices; partial y [S, D] goes out in bf16 and the host sums
  the 8 per-core partials.
"""

import math
import numpy as np
import ml_dtypes

import concourse.bass as bass
import concourse.mybir as mybir
import concourse.tile as tile
from concourse import bacc
from concourse.bass_utils import run_bass_kernel_spmd

B, S, D, H, HD = 1, 2048, 4096, 32, 128
NCORES = 8
HL = H // NCORES          # 4 heads per core
DL = HL * HD              # 512 local head dims
NT = S // 128             # 16 seq tiles of 128
NCH = S // 512            # 4 seq chunks of 512
KD = D // 128             # 32 contraction tiles
SCALE = 1.0 / math.sqrt(HD)
F32 = mybir.dt.float32
BF16 = mybir.dt.bfloat16
MUL = mybir.AluOpType.mult
ADD = mybir.AluOpType.add
SUB = mybir.AluOpType.subtract
EXP = mybir.ActivationFunctionType.Exp

BFNP = ml_dtypes.bfloat16

_CACHE = {}


def _build():
    nc = bacc.Bacc(None, target_bir_lowering=False, debug=False)
    xT_t = nc.dram_tensor("xT", [D, S], BF16, kind="ExternalInput")
    cos_t = nc.dram_tensor("cosT", [128, S], F32, kind="ExternalInput")
    sin_t = nc.dram_tensor("sinT", [128, S], F32, kind="ExternalInput")
    wq_t = nc.dram_tensor("wq", [D, DL], BF16, kind="ExternalInput")
    wk_t = nc.dram_tensor("wk", [D, DL], BF16, kind="ExternalInput")
    wv_t = nc.dram_tensor("wv", [D, DL], BF16, kind="ExternalInput")
    wo_t = nc.dram_tensor("wo", [DL, D], BF16, kind="ExternalInput")
    mk_t = nc.dram_tensor("maskT", [128, 128], BF16, kind="ExternalInput")
    y_t = nc.dram_tensor("y", [S, D], BF16, kind="ExternalOutput")

    xT_r = xT_t.ap().rearrange("(a r) s -> r a s", r=128)   # [128, 32, 2048]
    wq_r = wq_t.ap().rearrange("(a r) n -> r a n", r=128)   # [128, 32, 512]
    wk_r = wk_t.ap().rearrange("(a r) n -> r a n", r=128)
    wv_r = wv_t.ap().rearrange("(a r) n -> r a n", r=128)

    with tile.TileContext(nc) as tc:
        with (
            tc.tile_pool(name="pers", bufs=1) as pers,
            tc.tile_pool(name="xp", bufs=1) as xp,
            tc.tile_pool(name="wp", bufs=4) as wp,
            tc.tile_pool(name="rt", bufs=3) as rt,
            tc.tile_pool(name="rtc", bufs=1) as rtc,
            tc.tile_pool(name="ptp", bufs=1) as ptp,
            tc.tile_pool(name="atp", bufs=8) as atp,
            tc.tile_pool(name="rip", bufs=2) as rip,
            tc.tile_pool(name="yp", bufs=4) as yp,
            tc.tile_pool(name="ppj", bufs=1, space="PSUM") as ppj,
            tc.tile_pool(name="psc", bufs=2, space="PSUM") as psc,
            tc.tile_pool(name="prs", bufs=1, space="PSUM") as prs,
            tc.tile_pool(name="pat", bufs=1, space="PSUM") as pat,
            tc.tile_pool(name="pyo", bufs=2, space="PSUM") as pyo,
        ):
            qT = pers.tile([128, HL, S], BF16)      # [hd, h, s]
            kT = pers.tile([128, HL, S], BF16)
            vS = pers.tile([128, NT, DL], BF16)     # [s%128, s//128, dl]
            woT = pers.tile([128, HL, D], BF16)     # [dl%128, dl//128, o]
            cos2 = pers.tile([128, S], F32)   # cos duplicated in both halves
            sin2 = pers.tile([128, S], F32)
            maskT = pers.tile([128, 128], BF16)
            ones = pers.tile([128, 128], BF16)
            sign = pers.tile([128, 1], F32)   # -1 in rows 0:64, +1 in rows 64:128

            # startup loads off the critical path: cos/sin/mask/wo go on the
            # gpsimd queue; x + first weights stream on the sync queue first
            nc.gpsimd.dma_start(out=cos2[:], in_=cos_t[:, :])
            nc.gpsimd.dma_start(out=sin2[:], in_=sin_t[:, :])
            nc.gpsimd.dma_start(out=maskT[:], in_=mk_t[:, :])
            for h in range(HL):
                nc.gpsimd.dma_start(out=woT[:, h, :], in_=wo_t[h * 128:(h + 1) * 128, :])
            nc.vector.memset(ones[:], 1.0)
            nc.vector.memset(sign[0:64, :], -1.0)
            nc.vector.memset(sign[64:128, :], 1.0)

            for c in range(NCH):
                cs = slice(c * 512, (c + 1) * 512)
                g = c

                # ---- x chunk load (xT is read once total) ----
                xc = xp.tile([128, KD, 512], BF16, tag="xc")
                for q4 in range(KD // 4):
                    nc.sync.dma_start(
                        out=xc[:, q4 * 4:(q4 + 1) * 4, :],
                        in_=xT_r[:, q4 * 4:(q4 + 1) * 4, cs],
                    )

                # ---- q/k projections + fused rope, 2 head-pair passes ----
                for w_r, dstT in ((wq_r, qT), (wk_r, kT)):
                    for pA in range(2):
                        ph = slice(pA * 256, (pA + 1) * 256)
                        ps = ppj.tile([128, 1024], F32, tag="pj")
                        for q4 in range(KD // 4):
                            wt = wp.tile([128, 4, 256], BF16, tag="w")
                            nc.sync.dma_start(out=wt[:], in_=w_r[:, q4 * 4:(q4 + 1) * 4, ph])
                            for kk in range(4):
                                k = q4 * 4 + kk
                                for mm in range(2):
                                    nc.tensor.matmul(
                                        ps[:, mm * 512:(mm + 1) * 512],
                                        wt[:, kk, mm * 128:(mm + 1) * 128],
                                        xc[:, k, :],
                                        start=(k == 0), stop=(k == KD - 1),
                                    )
                        # rope evac: one ACT copy pc releases the PSUM banks;
                        # pcs = pair-swapped copy (DVE partition-shift), then per
                        # head u = pc*cos, w2 = (pcs*sign)*sin, dst = u + w2.
                        pc = rtc.tile([128, 1024], F32, tag="pc")
                        pcs = rtc.tile([128, 1024], F32, tag="pcs")
                        nc.scalar.copy(out=pc[:], in_=ps[:])
                        nc.vector.tensor_copy(out=pcs[0:64, :], in_=pc[64:128, :])
                        nc.vector.tensor_copy(out=pcs[64:128, :], in_=pc[0:64, :])
                        for mm in range(2):
                            m = 2 * pA + mm
                            hs = slice(mm * 512, (mm + 1) * 512)
                            u = rt.tile([128, 512], F32, tag="ra")
                            w2 = rt.tile([128, 512], F32, tag="rb")
                            nc.vector.tensor_tensor(out=u[:], in0=pc[:, hs], in1=cos2[:, cs], op=MUL)
                            nc.vector.scalar_tensor_tensor(out=w2[:], in0=pcs[:, hs], scalar=sign[:],
                                                           in1=sin2[:, cs], op0=MUL, op1=MUL)
                            nc.gpsimd.tensor_tensor(out=dstT[:, m, cs], in0=u[:], in1=w2[:], op=ADD)

                # ---- v projection, 2 s-tile-pair passes ----
                for pA in range(2):
                    ps = ppj.tile([128, 1024], F32, tag="pj")
                    for k2 in range(KD // 2):
                        wt = wp.tile([128, 2, 512], BF16, tag="wv")
                        nc.sync.dma_start(out=wt[:], in_=wv_r[:, k2 * 2:(k2 + 1) * 2, :])
                        for kk in range(2):
                            k = k2 * 2 + kk
                            for jj in range(2):
                                j = 2 * pA + jj
                                nc.tensor.matmul(
                                    ps[:, jj * 512:(jj + 1) * 512],
                                    xc[:, k, j * 128:(j + 1) * 128],
                                    wt[:, kk, :],
                                    start=(k == 0), stop=(k == KD - 1),
                                )
                    psr = ps.rearrange("p (j n) -> p j n", n=512)
                    nc.vector.tensor_copy(
                        out=vS[:, 4 * c + 2 * pA:4 * c + 2 * pA + 2, :],
                        in_=psr[:, 0:2, :],
                    )

                # ---- causal attention for q-block g (512 queries) ----
                nsk = 4 * g + 4
                gs = slice(g * 512, (g + 1) * 512)
                attn_g = []
                for h in range(HL):
                    PTt = ptp.tile([128, NT, 512], BF16, tag="pt")
                    rs = prs.tile([128, 512], F32, tag="rs")
                    at = pat.tile([128, 512], F32, tag="at")
                    for t in range(nsk):
                        sc = psc.tile([128, 512], F32, tag="sc")
                        j0 = t - 4 * g  # >=0 only in the diagonal group
                        lo = max(j0, 0) * 128
                        nc.tensor.matmul(
                            sc[:, lo:512],
                            kT[:, h, t * 128:(t + 1) * 128],
                            qT[:, h, g * 512 + lo:(g + 1) * 512],
                            start=True, stop=True,
                        )
                        nc.scalar.activation(PTt[:, t, lo:512], sc[:, lo:512], EXP, scale=SCALE)
                        if j0 >= 0:
                            nc.vector.tensor_tensor(
                                out=PTt[:, t, lo:lo + 128],
                                in0=PTt[:, t, lo:lo + 128],
                                in1=maskT[:], op=MUL,
                            )
                        nc.tensor.matmul(rs[:, lo:512], ones[:], PTt[:, t, lo:512],
                                         start=(t == 0), stop=(t == nsk - 1))
                        nc.tensor.matmul(at[:, lo:512], vS[:, t, h * 128:(h + 1) * 128],
                                         PTt[:, t, lo:512],
                                         start=(t == 0), stop=(t == nsk - 1))
                    ri = rip.tile([128, 512], F32, tag="ri")
                    nc.vector.reciprocal(ri[:], rs[:])
                    an = atp.tile([128, 512], BF16, tag="attn")
                    nc.vector.tensor_tensor(out=an[:], in0=at[:], in1=ri[:], op=MUL)
                    attn_g.append(an)

                # ---- wo partial for this q-block ----
                for j in range(4):
                    st = 4 * g + j
                    for oc in range(8):
                        yps = pyo.tile([128, 512], F32, tag="yo")
                        for h in range(HL):
                            nc.tensor.matmul(
                                yps[:],
                                attn_g[h][:, j * 128:(j + 1) * 128],
                                woT[:, h, oc * 512:(oc + 1) * 512],
                                start=(h == 0), stop=(h == HL - 1),
                            )
                        ysb = yp.tile([128, 512], BF16, tag="ysb")
                        if oc % 2 == 0:
                            nc.vector.tensor_copy(out=ysb[:], in_=yps[:])
                        else:
                            nc.scalar.copy(out=ysb[:], in_=yps[:])
                        nc.sync.dma_start(
                            out=y_t[st * 128:(st + 1) * 128, oc * 512:(oc + 1) * 512],
                            in_=ysb[:],
                        )

    nc.compile()
    return nc


def _prep_inputs(x, freqs, wq, wk, wv, wo):
    x2 = np.asarray(x, dtype=np.float32).reshape(S, D)
    xT = np.ascontiguousarray(x2.T).astype(BFNP)
    f = np.asarray(freqs, dtype=np.float32)
    c64 = np.cos(f).T.astype(np.float32)   # [64, S]
    s64 = np.sin(f).T.astype(np.float32)
    cosT = np.ascontiguousarray(np.concatenate([c64, c64], axis=0))  # [128, S]
    sinT = np.ascontiguousarray(np.concatenate([s64, s64], axis=0))
    # pair permutation: evens then odds within each head's 128 rows
    perm = np.concatenate([np.arange(0, HD, 2), np.arange(1, HD, 2)])
    i = np.arange(128)
    maskT = (i[:, None] <= i[None, :]).astype(BFNP)  # keep sk <= sq
    in_maps = []
    for c in range(NCORES):
        sl = slice(c * DL, (c + 1) * DL)
        wq_c = wq[sl, :].reshape(HL, HD, D)[:, perm, :].reshape(DL, D)
        wk_c = wk[sl, :].reshape(HL, HD, D)[:, perm, :].reshape(DL, D)
        in_maps.append({
            "xT": xT,
            "cosT": cosT,
            "sinT": sinT,
            "wq": np.ascontiguousarray(wq_c.T).astype(BFNP),
            "wk": np.ascontiguousarray(wk_c.T).astype(BFNP),
            "wv": np.ascontiguousarray(wv[sl, :].T).astype(BFNP),
            "wo": np.ascontiguousarray(wo[:, sl].T).astype(BFNP),
            "maskT": maskT,
        })
    return in_maps


def _run(inputs, trace=False):
    if "nc" not in _CACHE:
        _CACHE["nc"] = _build()
    nc = _CACHE["nc"]
    in_maps = _prep_inputs(**inputs)
    res = run_bass_kernel_spmd(nc, in_maps, core_ids=list(range(NCORES)), trace=trace)
    y = np.zeros((S, D), dtype=np.float64)
    for c in range(NCORES):
        y += res.results[c]["y"].astype(np.float64)
    return y.astype(np.float32).reshape(B, S, D), res.exec_time_ns


def kernel(**inputs):
    y, _ = _run(inputs, trace=False)
    return y
